# revision 8
# baseline (speedup 1.0000x reference)
"""Cross-attention Trainium2 kernel (nn_CrossAttention_8486855377137).

Sharding (8 cores): core c = (batch b = c//2, head-group g = c%2).
Each core handles one batch and 4 of the 8 heads (Q/K/V projections
column-sharded by head, wo row-sharded). Full softmax over S on device;
host sums the two partial wo outputs per batch and adds wo bias.

Device dataflow (all transposed-world, zero on-device transposes):
  QT[gE=256, T]   = wqT.T @ xT     (+bq)
  KT[gE, S]       = wkT.T @ ctxT   (+bk)
  V[S, gE]        = ctxT.T @ wvT   (+bv)     (stored padded with ones col per head)
  scoresT[s,t]    = KT_h.T-slice matmuls, 2 heads row-packed per pair (K=64)
  expS            = exp(0.125 * scoresT)     (ScalarE, PSUM->SBUF bf16)
  O'[65, t]       = sum_s V'_h[s].T @ expS   (ones col -> row 64 = denominator)
  Ocat            = O'[0:64] * (1/den)       (normalized, bf16)
  yT[512, T]      = woT.T @ Ocat             (partial over head-group, fp32)
"""

import numpy as np
import ml_dtypes

import concourse.bass as bass
import concourse.bacc as bacc
import concourse.tile as tile
import concourse.mybir as mybir
from concourse.bass_utils import run_bass_kernel_spmd

BF16 = mybir.dt.bfloat16
F32 = mybir.dt.float32
EXP = mybir.ActivationFunctionType.Exp
ADD = mybir.AluOpType.add
MULT = mybir.AluOpType.mult
NPBF16 = ml_dtypes.bfloat16

# Problem constants (hardcoded per contract)
B, T, S = 4, 2048, 4096
E, KV = 512, 2048
H, D = 8, 64
GE = 256          # head-group embed width (4 heads x 64)
SCALE = D ** -0.5  # 0.125

N_CORES = 8


def _build_nc():
    nc = bacc.Bacc("TRN2", target_bir_lowering=False, debug=False)

    ctxT = nc.dram_tensor("ctxT", [KV, S], BF16, kind="ExternalInput")
    xT = nc.dram_tensor("xT", [E, T], BF16, kind="ExternalInput")
    wqT = nc.dram_tensor("wqT", [E, GE], BF16, kind="ExternalInput")
    wkT = nc.dram_tensor("wkT", [KV, GE], BF16, kind="ExternalInput")
    wvT = nc.dram_tensor("wvT", [KV, GE], BF16, kind="ExternalInput")
    woT = nc.dram_tensor("woT", [GE, E], BF16, kind="ExternalInput")
    bq = nc.dram_tensor("bq", [GE], F32, kind="ExternalInput")
    bk = nc.dram_tensor("bk", [GE], F32, kind="ExternalInput")
    bv = nc.dram_tensor("bv", [GE], F32, kind="ExternalInput")
    yT = nc.dram_tensor("yT", [E, T], F32, kind="ExternalOutput")

    with tile.TileContext(nc) as tc:
        _kernel_body(tc, nc, ctxT, xT, wqT, wkT, wvT, woT, bq, bk, bv, yT)
    nc.compile()
    return nc


def _kernel_body(tc, nc, ctxT, xT, wqT, wkT, wvT, woT, bq, bk, bv, yT):
    P = 128
    NT = T // 512        # 4 t-chunks
    NSC = S // P         # 32 s-chunks of 128
    NST = S // P         # 32 s-tiles for V
    KV_C = KV // P       # 16 contraction chunks for K/V proj
    E_C = E // P         # 4 contraction chunks for Q proj

    wts = tc.alloc_tile_pool(name="wts", bufs=1)
    persist = tc.alloc_tile_pool(name="persist", bufs=1)

    # ---- constant / weight loads ----
    wqT_sb = wts.tile([P, E_C, GE], BF16, tag="wqT")
    nc.sync.dma_start(wqT_sb, wqT.rearrange("(c p) m -> p c m", p=P))
    wkT_sb = wts.tile([P, KV_C, GE], BF16, tag="wkT")
    nc.sync.dma_start(wkT_sb, wkT.rearrange("(c p) m -> p c m", p=P))
    wvT_sb = wts.tile([P, KV_C, GE], BF16, tag="wvT")
    nc.sync.dma_start(wvT_sb, wvT.rearrange("(c p) m -> p c m", p=P))
    woT_sb = wts.tile([P, 2, E], BF16, tag="woT")
    nc.sync.dma_start(woT_sb, woT.rearrange("(c p) m -> p c m", p=P))
    xT_sb = wts.tile([P, E_C, T], BF16, tag="xT")
    nc.sync.dma_start(xT_sb, xT.rearrange("(c p) t -> p c t", p=P))

    bq_sb = wts.tile([P, 2], F32, tag="bq")
    nc.sync.dma_start(bq_sb, bq.rearrange("(c p) -> p c", p=P))
    bk_sb = wts.tile([P, 2], F32, tag="bk")
    nc.sync.dma_start(bk_sb, bk.rearrange("(c p) -> p c", p=P))
    # bv broadcast to all 128 partitions (fp32), used along free dim of V
    bv_bc = wts.tile([P, GE], F32, tag="bv_bc")
    bv_ap = bv.ap()
    bv_bcast_src = bass.AP(tensor=bv_ap.tensor, offset=bv_ap.offset,
                           ap=[[0, P]] + list(bv_ap.ap))
    nc.gpsimd.dma_start(out=bv_bc, in_=bv_bcast_src)

    # ---- persistent activation tiles ----
    QT_sb = [persist.tile([P, T], BF16, tag=f"QT{c}", name=f"QT{c}") for c in range(2)]
    KT_sb = [persist.tile([P, S], BF16, tag=f"KT{c}", name=f"KT{c}") for c in range(2)]
    # V padded: per s-tile [128, 4*65]; col h*65+64 holds ones
    VP = [persist.tile([P, 4 * 65], BF16, tag=f"VP{i}", name=f"VP{i}") for i in range(NST)]
    for i in range(NST):
        ones_view = VP[i].rearrange("p (h e) -> p h e", e=65)[:, :, 64:65]
        nc.vector.memset(ones_view, 1.0)
    OcatT = [persist.tile([P, T], BF16, tag=f"Ocat{c}", name=f"Ocat{c}") for c in range(2)]

    # ================= Phase 1: projections =================
    with tc.tile_pool(name="p1psum", bufs=2, space="PSUM") as p1ps, \
         tc.tile_pool(name="ctxpool", bufs=2) as ctxpool:

        # QT
        for m in range(2):
            for t in range(NT):
                ps = p1ps.tile([P, 512], F32, tag="qps")
                for c in range(E_C):
                    nc.tensor.matmul(
                        ps, wqT_sb[:, c, m * P:(m + 1) * P],
                        xT_sb[:, c, t * 512:(t + 1) * 512],
                        start=(c == 0), stop=(c == E_C - 1))
                nc.vector.tensor_scalar_add(
                    QT_sb[m][:, t * 512:(t + 1) * 512], ps, bq_sb[:, m:m + 1])

        # KT + V, streaming ctxT in 4 s-groups of 1024
        for sg in range(4):
            ctx_t = ctxpool.tile([P, KV_C, 1024], BF16, tag="ctx")
            nc.sync.dma_start(
                ctx_t,
                ctxT.rearrange("(c p) s -> p c s", p=P)[:, :, sg * 1024:(sg + 1) * 1024])
            for m in range(2):
                for n in range(2):
                    ps = p1ps.tile([P, 512], F32, tag="kps")
                    for c in range(KV_C):
                        nc.tensor.matmul(
                            ps, wkT_sb[:, c, m * P:(m + 1) * P],
                            ctx_t[:, c, n * 512:(n + 1) * 512],
                            start=(c == 0), stop=(c == KV_C - 1))
                    nc.vector.tensor_scalar_add(
                        KT_sb[m][:, sg * 1024 + n * 512: sg * 1024 + (n + 1) * 512],
                        ps, bk_sb[:, m:m + 1])
            for st in range(8):
                ps = p1ps.tile([P, GE], F32, tag="vps")
                for c in range(KV_C):
                    nc.tensor.matmul(
                        ps, ctx_t[:, c, st * P:(st + 1) * P], wvT_sb[:, c, :],
                        start=(c == 0), stop=(c == KV_C - 1))
                vp = VP[sg * 8 + st]
                nc.vector.tensor_tensor(
                    vp.rearrange("p (h e) -> p h e", e=65)[:, :, 0:64],
                    ps.rearrange("p (h e) -> p h e", e=64),
                    bv_bc.rearrange("p (h e) -> p h e", e=64),
                    ADD)

    # ================= Phase 2: attention + out-proj =================
    with tc.tile_pool(name="aps", bufs=1, space="PSUM") as aps, \
         tc.tile_pool(name="espool", bufs=3) as espool, \
         tc.tile_pool(name="npool", bufs=2) as npool, \
         tc.tile_pool(name="dramp", bufs=2, space="DRAM") as dramp, \
         tc.tile_pool(name="ystg", bufs=2) as ystg:

        def attn_unit(c2, t):
            h0, h1 = 2 * c2, 2 * c2 + 1
            o_ps = [aps.tile([P, 512], F32, tag=f"o{j}", name=f"ops{j}") for j in range(2)]
            for s in range(NSC):
                slab = aps.tile([P, 1024], F32, tag="slab", bufs=2)
                nc.tensor.matmul(
                    slab[:, 0:512],
                    KT_sb[c2][0:64, s * P:(s + 1) * P],
                    QT_sb[c2][0:64, t * 512:(t + 1) * 512],
                    start=True, stop=True, skip_group_check=True)
                nc.tensor.matmul(
                    slab[:, 512:1024],
                    KT_sb[c2][64:128, s * P:(s + 1) * P],
                    QT_sb[c2][64:128, t * 512:(t + 1) * 512],
                    start=True, stop=True, skip_group_check=True)
                es = espool.tile([P, 1024], BF16, tag="es")
                nc.scalar.activation(es, slab, EXP, scale=SCALE)
                for j, h in enumerate((h0, h1)):
                    nc.tensor.matmul(
                        o_ps[j][:65],
                        VP[s][:, h * 65:(h + 1) * 65],
                        es[:, j * 512:(j + 1) * 512],
                        start=(s == 0), stop=(s == NSC - 1),
                        skip_group_check=True)
            # normalize and evict
            for j in range(2):
                ps = o_ps[j]
                inv = npool.tile([65, 512], F32, tag="inv")
                nc.vector.reciprocal(inv[64:65, :], ps[64:65, :])
                # bounce via DRAM to broadcast along partitions
                dscr = dramp.tile([1, 512], F32, tag="dscr")
                nc.sync.dma_start(dscr, inv[64:65, :])
                bc = npool.tile([64, 512], F32, tag="bc")
                dap = dscr
                bcast_src = bass.AP(tensor=dap.tensor, offset=dap.offset,
                                    ap=[[0, 64]] + list(dap.ap[1:]))
                nc.gpsimd.dma_start(out=bc, in_=bcast_src)
                if j == 0:
                    nc.vector.tensor_tensor(
                        OcatT[c2][0:64, t * 512:(t + 1) * 512],
                        ps[0:64, :], bc, MULT)
                else:
                    stg = npool.tile([64, 512], BF16, tag="stg")
                    nc.vector.tensor_tensor(stg, ps[0:64, :], bc, MULT)
                    nc.sync.dma_start(
                        OcatT[c2][64:128, t * 512:(t + 1) * 512], stg)

        def yproj(t):
            yT_r = yT.rearrange("(m p) t -> p m t", p=P)
            for m in range(E // P):
                ps = aps.tile([P, 512], F32, tag="yps", bufs=2)
                for c2 in range(2):
                    nc.tensor.matmul(
                        ps, woT_sb[:, c2, m * P:(m + 1) * P],
                        OcatT[c2][:, t * 512:(t + 1) * 512],
                        start=(c2 == 0), stop=(c2 == 1))
                yo = ystg.tile([P, 512], F32, tag="yo")
                nc.vector.tensor_copy(yo, ps)
                nc.sync.dma_start(yT_r[:, m, t * 512:(t + 1) * 512], yo)

        # emission order: delay yproj(t) by one unit so eviction tails are done
        for t in range(NT):
            for c2 in range(2):
                attn_unit(c2, t)
                if t > 0 and c2 == 0:
                    yproj(t - 1)
        yproj(NT - 1)

    persist.release()
    wts.release()


_NC_CACHE = None
LAST_RESULT = None


def _get_nc():
    global _NC_CACHE
    if _NC_CACHE is None:
        _NC_CACHE = _build_nc()
    return _NC_CACHE


def kernel(x, context, wq_w, wq_b, wk_w, wk_b, wv_w, wv_b, wo_w, wo_b):
    x = np.asarray(x)
    context = np.asarray(context)
    nc = _get_nc()

    ctxT = [np.ascontiguousarray(context[b].T).astype(NPBF16) for b in range(B)]
    xT = [np.ascontiguousarray(x[b].T).astype(NPBF16) for b in range(B)]

    in_maps = []
    for c in range(N_CORES):
        b, g = c // 2, c % 2
        sl = slice(g * GE, (g + 1) * GE)
        in_maps.append({
            "ctxT": ctxT[b],
            "xT": xT[b],
            "wqT": np.ascontiguousarray(np.asarray(wq_w)[sl, :].T).astype(NPBF16),
            "wkT": np.ascontiguousarray(np.asarray(wk_w)[sl, :].T).astype(NPBF16),
            "wvT": np.ascontiguousarray(np.asarray(wv_w)[sl, :].T).astype(NPBF16),
            "woT": np.ascontiguousarray(np.asarray(wo_w)[:, sl].T).astype(NPBF16),
            "bq": np.ascontiguousarray(np.asarray(wq_b)[sl]).astype(np.float32),
            "bk": np.ascontiguousarray(np.asarray(wk_b)[sl]).astype(np.float32),
            "bv": np.ascontiguousarray(np.asarray(wv_b)[sl]).astype(np.float32),
        })

    res = run_bass_kernel_spmd(nc, in_maps, core_ids=list(range(N_CORES)))
    global LAST_RESULT
    LAST_RESULT = res
    outs = res.results

    wo_b = np.asarray(wo_b, dtype=np.float32)
    y = np.empty((B, T, E), dtype=np.float32)
    for b in range(B):
        yt = outs[2 * b]["yT"] + outs[2 * b + 1]["yT"]
        y[b] = yt.T + wo_b
    return y


# revision 10
# speedup vs baseline: 1.1148x; 1.1148x over previous
"""Cross-attention Trainium2 kernel (nn_CrossAttention_8486855377137).

Sharding (8 cores): core c = (batch b = c//2, head-group g = c%2).
Each core handles one batch and 4 of the 8 heads (Q/K/V projections
column-sharded by head, wo row-sharded). Full softmax over S on device;
host sums the two partial wo outputs per batch and adds wo bias.

Device dataflow (all transposed-world, zero on-device transposes):
  QT[gE=256, T]   = wqT.T @ xT     (+bq)
  KT[gE, S]       = wkT.T @ ctxT   (+bk)
  V[S, gE]        = ctxT.T @ wvT   (+bv)     (stored padded with ones col per head)
  scoresT[s,t]    = KT_h.T-slice matmuls, 2 heads row-packed per pair (K=64)
  expS            = exp(0.125 * scoresT)     (ScalarE, PSUM->SBUF bf16)
  O'[65, t]       = sum_s V'_h[s].T @ expS   (ones col -> row 64 = denominator)
  Ocat            = O'[0:64] * (1/den)       (normalized, bf16)
  yT[512, T]      = woT.T @ Ocat             (partial over head-group, fp32)
"""

import numpy as np
import ml_dtypes

import concourse.bass as bass
import concourse.bacc as bacc
import concourse.tile as tile
import concourse.mybir as mybir
from concourse.bass_utils import run_bass_kernel_spmd

BF16 = mybir.dt.bfloat16
F32 = mybir.dt.float32
EXP = mybir.ActivationFunctionType.Exp
ADD = mybir.AluOpType.add
MULT = mybir.AluOpType.mult
NPBF16 = ml_dtypes.bfloat16

# Problem constants (hardcoded per contract)
B, T, S = 4, 2048, 4096
E, KV = 512, 2048
H, D = 8, 64
GE = 256          # head-group embed width (4 heads x 64)
SCALE = D ** -0.5  # 0.125

N_CORES = 8


def _build_nc():
    nc = bacc.Bacc("TRN2", target_bir_lowering=False, debug=False)

    ctxT = nc.dram_tensor("ctxT", [KV, S], BF16, kind="ExternalInput")
    xT = nc.dram_tensor("xT", [E, T], BF16, kind="ExternalInput")
    wqT = nc.dram_tensor("wqT", [E, GE], BF16, kind="ExternalInput")
    wkT = nc.dram_tensor("wkT", [KV, GE], BF16, kind="ExternalInput")
    wvT = nc.dram_tensor("wvT", [KV, GE], BF16, kind="ExternalInput")
    woT = nc.dram_tensor("woT", [GE, E], BF16, kind="ExternalInput")
    bq = nc.dram_tensor("bq", [GE], F32, kind="ExternalInput")
    bk = nc.dram_tensor("bk", [GE], F32, kind="ExternalInput")
    bv = nc.dram_tensor("bv", [GE], F32, kind="ExternalInput")
    yT = nc.dram_tensor("yT", [E, T], F32, kind="ExternalOutput")

    with tile.TileContext(nc) as tc:
        _kernel_body(tc, nc, ctxT, xT, wqT, wkT, wvT, woT, bq, bk, bv, yT)
    nc.compile()
    return nc


def _kernel_body(tc, nc, ctxT, xT, wqT, wkT, wvT, woT, bq, bk, bv, yT):
    P = 128
    NT = T // 512        # 4 t-chunks
    NSC = S // P         # 32 s-chunks of 128
    NST = S // P         # 32 s-tiles for V
    KV_C = KV // P       # 16 contraction chunks for K/V proj
    E_C = E // P         # 4 contraction chunks for Q proj

    wts = tc.alloc_tile_pool(name="wts", bufs=1)
    persist = tc.alloc_tile_pool(name="persist", bufs=1)

    # ---- constant / weight loads ----
    wqT_sb = wts.tile([P, E_C, GE], BF16, tag="wqT")
    nc.sync.dma_start(wqT_sb, wqT.rearrange("(c p) m -> p c m", p=P))
    wkT_sb = wts.tile([P, KV_C, GE], BF16, tag="wkT")
    nc.sync.dma_start(wkT_sb, wkT.rearrange("(c p) m -> p c m", p=P))
    wvT_sb = wts.tile([P, KV_C, GE], BF16, tag="wvT")
    nc.sync.dma_start(wvT_sb, wvT.rearrange("(c p) m -> p c m", p=P))
    woT_sb = wts.tile([P, 2, E], BF16, tag="woT")
    nc.sync.dma_start(woT_sb, woT.rearrange("(c p) m -> p c m", p=P))
    xT_sb = wts.tile([P, E_C, T], BF16, tag="xT")
    nc.sync.dma_start(xT_sb, xT.rearrange("(c p) t -> p c t", p=P))

    bq_sb = wts.tile([P, 2], F32, tag="bq")
    nc.sync.dma_start(bq_sb, bq.rearrange("(c p) -> p c", p=P))
    bk_sb = wts.tile([P, 2], F32, tag="bk")
    nc.sync.dma_start(bk_sb, bk.rearrange("(c p) -> p c", p=P))
    # bv broadcast to all 128 partitions (fp32), used along free dim of V
    bv_bc = wts.tile([P, GE], F32, tag="bv_bc")
    bv_ap = bv.ap()
    bv_bcast_src = bass.AP(tensor=bv_ap.tensor, offset=bv_ap.offset,
                           ap=[[0, P]] + list(bv_ap.ap))
    nc.gpsimd.dma_start(out=bv_bc, in_=bv_bcast_src)

    # ---- persistent activation tiles ----
    QT_sb = [persist.tile([P, T], BF16, tag=f"QT{c}", name=f"QT{c}") for c in range(2)]
    KT_sb = [persist.tile([P, S], BF16, tag=f"KT{c}", name=f"KT{c}") for c in range(2)]
    # V padded: per s-tile [128, 4*65]; col h*65+64 holds ones
    VP = [persist.tile([P, 4 * 65], BF16, tag=f"VP{i}", name=f"VP{i}") for i in range(NST)]
    for i in range(NST):
        ones_view = VP[i].rearrange("p (h e) -> p h e", e=65)[:, :, 64:65]
        nc.vector.memset(ones_view, 1.0)
    OcatT = [persist.tile([P, T], BF16, tag=f"Ocat{c}", name=f"Ocat{c}") for c in range(2)]

    # ================= Phase 1: projections =================
    with tc.tile_pool(name="p1psum", bufs=2, space="PSUM") as p1ps, \
         tc.tile_pool(name="ctxpool", bufs=2) as ctxpool:

        # QT
        for m in range(2):
            for t in range(NT):
                ps = p1ps.tile([P, 512], F32, tag="qps")
                for c in range(E_C):
                    nc.tensor.matmul(
                        ps, wqT_sb[:, c, m * P:(m + 1) * P],
                        xT_sb[:, c, t * 512:(t + 1) * 512],
                        start=(c == 0), stop=(c == E_C - 1))
                nc.vector.tensor_scalar_add(
                    QT_sb[m][:, t * 512:(t + 1) * 512], ps, bq_sb[:, m:m + 1])

        # KT + V, streaming ctxT in 4 s-groups of 1024
        for sg in range(4):
            ctx_t = ctxpool.tile([P, KV_C, 1024], BF16, tag="ctx")
            nc.sync.dma_start(
                ctx_t,
                ctxT.rearrange("(c p) s -> p c s", p=P)[:, :, sg * 1024:(sg + 1) * 1024])
            for m in range(2):
                for n in range(2):
                    ps = p1ps.tile([P, 512], F32, tag="kps")
                    for c in range(KV_C):
                        nc.tensor.matmul(
                            ps, wkT_sb[:, c, m * P:(m + 1) * P],
                            ctx_t[:, c, n * 512:(n + 1) * 512],
                            start=(c == 0), stop=(c == KV_C - 1))
                    nc.vector.tensor_scalar_add(
                        KT_sb[m][:, sg * 1024 + n * 512: sg * 1024 + (n + 1) * 512],
                        ps, bk_sb[:, m:m + 1])
            for st in range(8):
                ps = p1ps.tile([P, GE], F32, tag="vps")
                for c in range(KV_C):
                    nc.tensor.matmul(
                        ps, ctx_t[:, c, st * P:(st + 1) * P], wvT_sb[:, c, :],
                        start=(c == 0), stop=(c == KV_C - 1))
                vp = VP[sg * 8 + st]
                nc.vector.tensor_tensor(
                    vp.rearrange("p (h e) -> p h e", e=65)[:, :, 0:64],
                    ps.rearrange("p (h e) -> p h e", e=64),
                    bv_bc.rearrange("p (h e) -> p h e", e=64),
                    ADD)

    # ================= Phase 2: attention + out-proj =================
    with tc.tile_pool(name="aps", bufs=1, space="PSUM") as aps, \
         tc.tile_pool(name="espool", bufs=3) as espool, \
         tc.tile_pool(name="npool", bufs=2) as npool, \
         tc.tile_pool(name="dramp", bufs=2, space="DRAM") as dramp, \
         tc.tile_pool(name="ystg", bufs=2) as ystg:

        def attn_unit(c2, t):
            h0, h1 = 2 * c2, 2 * c2 + 1
            o_ps = [aps.tile([P, 512], F32, tag=f"o{j}", name=f"ops{j}", bufs=2)
                    for j in range(2)]
            for s in range(NSC):
                slab = aps.tile([P, 1024], F32, tag="slab", bufs=2)
                nc.tensor.matmul(
                    slab[:, 0:512],
                    KT_sb[c2][0:64, s * P:(s + 1) * P],
                    QT_sb[c2][0:64, t * 512:(t + 1) * 512],
                    start=True, stop=True, skip_group_check=True)
                nc.tensor.matmul(
                    slab[:, 512:1024],
                    KT_sb[c2][64:128, s * P:(s + 1) * P],
                    QT_sb[c2][64:128, t * 512:(t + 1) * 512],
                    start=True, stop=True, skip_group_check=True)
                es = espool.tile([P, 1024], BF16, tag="es")
                nc.scalar.activation(es, slab, EXP, scale=SCALE)
                for j, h in enumerate((h0, h1)):
                    nc.tensor.matmul(
                        o_ps[j][:65],
                        VP[s][:, h * 65:(h + 1) * 65],
                        es[:, j * 512:(j + 1) * 512],
                        start=(s == 0), stop=(s == NSC - 1),
                        skip_group_check=True)
            # fast unnormalized eviction (frees PSUM quickly), then
            # normalization off the PE critical path
            for j in range(2):
                ps = o_ps[j]
                ou = npool.tile([65, 512], F32, tag="ou", bufs=4)
                nc.vector.tensor_copy(ou, ps[:65, :])
                # den -> DRAM -> broadcast to 64 partitions
                dscr = dramp.tile([1, 512], F32, tag="dscr")
                nc.sync.dma_start(dscr, ou[64:65, :])
                bc = npool.tile([64, 512], F32, tag="bc")
                bcast_src = bass.AP(tensor=dscr.tensor, offset=dscr.offset,
                                    ap=[[0, 64]] + list(dscr.ap[1:]))
                nc.gpsimd.dma_start(out=bc, in_=bcast_src)
                inv = npool.tile([64, 512], F32, tag="inv")
                nc.vector.reciprocal(inv, bc)
                if j == 0:
                    nc.vector.tensor_tensor(
                        OcatT[c2][0:64, t * 512:(t + 1) * 512],
                        ou[0:64, :], inv, MULT)
                else:
                    stg = npool.tile([64, 512], BF16, tag="stg")
                    nc.vector.tensor_tensor(stg, ou[0:64, :], inv, MULT)
                    nc.sync.dma_start(
                        OcatT[c2][64:128, t * 512:(t + 1) * 512], stg)

        def yproj(t):
            yT_r = yT.rearrange("(m p) t -> p m t", p=P)
            for m in range(E // P):
                ps = aps.tile([P, 1024], F32, tag="slab", bufs=2,
                              name=f"yps{m}")[:, 0:512]
                for c2 in range(2):
                    nc.tensor.matmul(
                        ps, woT_sb[:, c2, m * P:(m + 1) * P],
                        OcatT[c2][:, t * 512:(t + 1) * 512],
                        start=(c2 == 0), stop=(c2 == 1))
                yo = ystg.tile([P, 512], F32, tag="yo")
                nc.vector.tensor_copy(yo, ps)
                nc.sync.dma_start(yT_r[:, m, t * 512:(t + 1) * 512], yo)

        # emission order: delay yproj(t) by one unit so eviction tails are done
        for t in range(NT):
            for c2 in range(2):
                attn_unit(c2, t)
                if t > 0 and c2 == 0:
                    yproj(t - 1)
        yproj(NT - 1)

    persist.release()
    wts.release()


_NC_CACHE = None
LAST_RESULT = None


def _get_nc():
    global _NC_CACHE
    if _NC_CACHE is None:
        _NC_CACHE = _build_nc()
    return _NC_CACHE


def kernel(x, context, wq_w, wq_b, wk_w, wk_b, wv_w, wv_b, wo_w, wo_b):
    x = np.asarray(x)
    context = np.asarray(context)
    nc = _get_nc()

    ctxT = [np.ascontiguousarray(context[b].T).astype(NPBF16) for b in range(B)]
    xT = [np.ascontiguousarray(x[b].T).astype(NPBF16) for b in range(B)]

    in_maps = []
    for c in range(N_CORES):
        b, g = c // 2, c % 2
        sl = slice(g * GE, (g + 1) * GE)
        in_maps.append({
            "ctxT": ctxT[b],
            "xT": xT[b],
            "wqT": np.ascontiguousarray(np.asarray(wq_w)[sl, :].T).astype(NPBF16),
            "wkT": np.ascontiguousarray(np.asarray(wk_w)[sl, :].T).astype(NPBF16),
            "wvT": np.ascontiguousarray(np.asarray(wv_w)[sl, :].T).astype(NPBF16),
            "woT": np.ascontiguousarray(np.asarray(wo_w)[:, sl].T).astype(NPBF16),
            "bq": np.ascontiguousarray(np.asarray(wq_b)[sl]).astype(np.float32),
            "bk": np.ascontiguousarray(np.asarray(wk_b)[sl]).astype(np.float32),
            "bv": np.ascontiguousarray(np.asarray(wv_b)[sl]).astype(np.float32),
        })

    res = run_bass_kernel_spmd(nc, in_maps, core_ids=list(range(N_CORES)))
    global LAST_RESULT
    LAST_RESULT = res
    outs = res.results

    wo_b = np.asarray(wo_b, dtype=np.float32)
    y = np.empty((B, T, E), dtype=np.float32)
    for b in range(B):
        yt = outs[2 * b]["yT"] + outs[2 * b + 1]["yT"]
        y[b] = yt.T + wo_b
    return y
